# revision 49
# baseline (speedup 1.0000x reference)
"""Channel-attention block (qkv 1x1 conv -> C x C channel attention -> proj)
for Trainium2, data-parallel over batch across 8 NeuronCores.

Math (per batch element, x: [C, N] with C=512, N=9216):
  q = Wq x + bq 1^T,  k = Wk x + bk 1^T,  v = Wv x + bv 1^T
  S = (q k^T) / sqrt(C);  attn = softmax_rows(S);  y = Wp (attn v) + bp 1^T

Gram-matrix factorisation (~4.5 GFLOP/core instead of 29):
  G  = x x^T, xs = x 1
  S  = s*Wq G Wk^T + (s*Wq xs) bk^T + (s*bq)(Wk xs + N bk)^T
  E  = exp(S),  rs = 1/(E 1)
  Z  = Wp diag(rs) E Wv,  r = Wp (diag(rs) E bv) + bp
  y  = Z x + r 1^T

Design (the kernel is PE-stream-bound; everything else hides under it):
  * all matmuls in bf16 (fp32 PSUM): fixes the fp32r N<256 4x penalty,
    halves LDWEIGHTS (FWL), halves DMA; rel-err ~8e-3 vs the 2e-2 gate.
  * host uploads BOTH layouts of x, pre-tiled for 2-8KB-contiguous DMA
    lines: xT2 [128, N/128, C] feeds pass-1's G = x x^T directly (zero
    on-device transposes), x2 [128, CB, N] is SBUF-resident for pass 2.
  * one input DMA queue, ordered by criticality: 9 xT chunks first, the
    9 resident-x chunks strictly behind them (FIFO gating).
  * xs = x@1 via DVE-accumulated xT slabs + 4 tiny PE matmuls against a
    ones column (the only cross-partition reduce available).
  * chain is scheduled for the in-order PE queue + HAM clock-gate: G's
    lower-triangle transposes interleave with A's kd-steps, the two
    rank-1 S bias terms fuse into one rank-2 matmul per block, each P1
    group trails one S group behind its exp, r-matmuls cover the P1
    copies, and every GEMM handoff uses split PSUM pools (no WAW on a
    shared slot) with PSUM->SBUF copies split in consumption order.
  * y is stored bf16 and upcast on host.
Measured: ~110 us/iteration vs 202 us for the fp32r baseline (same
paired-slope harness), ~95% PE-pipe occupancy.
"""

import os as _os

import numpy as np
import ml_dtypes

import concourse.bass as bass
import concourse.bacc as bacc
import concourse.mybir as mybir
import concourse.tile as tile
from concourse.bass_utils import run_bass_kernel_spmd
from concourse.masks import make_identity

F32 = mybir.dt.float32
BF16 = mybir.dt.bfloat16
AX = mybir.AxisListType.X
AF = mybir.ActivationFunctionType

B = 8
C = 512
H = W = 96
N = H * W          # 9216
P = 128
CB = C // P        # 4 channel blocks
SCALE = 1.0 / float(np.sqrt(C))

XCH = 1024         # xT rows per pass-1 chunk
NXCH = N // XCH    # 9
SLABS = XCH // P   # 8
X2CH = 1024        # x cols per resident chunk
NX2 = N // X2CH    # 9


def _build_nc(repeat=1):
    nc = bacc.Bacc("TRN2", target_bir_lowering=False, debug=False, num_devices=B)

    # host-pretiled layouts: xT2[p, t, c] = x[c, t*128+p] (8KB contiguous
    # per partition per chunk), x2[p, kc, n] = x[kc*128+p, n] (2KB lines).
    xT_d = nc.declare_dram_parameter("xT2", [P, N // P, C], BF16, isOutput=False)
    x_d = nc.declare_dram_parameter("x2", [P, CB, N], BF16, isOutput=False)
    wqT_d = nc.declare_dram_parameter("wqT", [C, C], BF16, isOutput=False)
    wkT_d = nc.declare_dram_parameter("wkT", [C, C], BF16, isOutput=False)
    wpT_d = nc.declare_dram_parameter("wpT", [C, C], BF16, isOutput=False)
    wv_d = nc.declare_dram_parameter("wv", [C, C], BF16, isOutput=False)
    bqs_d = nc.declare_dram_parameter("bqs", [1, C], BF16, isOutput=False)
    bk_d = nc.declare_dram_parameter("bk", [1, C], BF16, isOutput=False)
    bkN_d = nc.declare_dram_parameter("bkN", [1, C], F32, isOutput=False)
    bv_d = nc.declare_dram_parameter("bv", [1, C], BF16, isOutput=False)
    bp_d = nc.declare_dram_parameter("bp", [1, C], F32, isOutput=False)
    y_d = nc.declare_dram_parameter("y", [C, N], BF16, isOutput=True)

    with tile.TileContext(nc) as tc:
        with (
            tc.tile_pool(name="consts", bufs=1) as consts,
        ):
            # ---- constants (loaded once, persist across repeats) --------
            # all on the gpsimd queue so the sync queue starts streaming
            # xT chunk 0 immediately at iteration start.
            wqT_sb = consts.tile([P, CB, C], BF16)
            nc.gpsimd.dma_start(
                out=wqT_sb, in_=wqT_d.rearrange("(kc p) i -> p kc i", p=P)
            )
            wkT_sb = consts.tile([P, CB, C], BF16)
            nc.gpsimd.dma_start(
                out=wkT_sb, in_=wkT_d.rearrange("(kc p) i -> p kc i", p=P)
            )
            wpT_sb = consts.tile([P, CB, C], BF16)
            nc.gpsimd.dma_start(
                out=wpT_sb, in_=wpT_d.rearrange("(kc p) i -> p kc i", p=P)
            )
            wv_sb = consts.tile([P, CB, C], BF16)
            nc.gpsimd.dma_start(
                out=wv_sb, in_=wv_d.rearrange("(kd p) a -> p kd a", p=P)
            )
            # rank-2 bias stacks: qs = (u2, s*bq), ub = (bk, u); constant
            # rows land once via DMA (engines can't write partition 1),
            # per-iteration rows are written below.
            qs_stack = consts.tile([2, C], BF16)
            nc.gpsimd.dma_start(out=qs_stack[1:2, :], in_=bqs_d[0:1, :])
            ub_stack = consts.tile([2, C], BF16)
            nc.gpsimd.dma_start(out=ub_stack[0:1, :], in_=bk_d[0:1, :])
            bkN_row = consts.tile([1, C], F32)
            nc.gpsimd.dma_start(out=bkN_row, in_=bkN_d[0:1, :])
            bv_bcast = consts.tile([P, C], BF16)
            _src = bv_d[0:1, :]
            nc.gpsimd.dma_start(
                out=bv_bcast,
                in_=bass.AP(tensor=_src.tensor, offset=_src.offset,
                            ap=[[0, P]] + _src.ap[1:]),
            )
            bp_col = consts.tile([P, CB], F32)
            nc.gpsimd.dma_start(
                out=bp_col, in_=bp_d[0:1, :].rearrange("o (kc p) -> (o p) kc", p=P)
            )
            ident32 = consts.tile([P, P], F32)
            make_identity(nc, ident32)
            identb = consts.tile([P, P], BF16)
            nc.vector.tensor_copy(identb, ident32)
            ones_col = consts.tile([P, 1], BF16)
            nc.vector.memset(ones_col, 1.0)

            def body(first_iter):
                itp_cm = tc.tile_pool(name="iter", bufs=1)
                itp = itp_cm.__enter__()

                # resident x tiles for pass 2 (DMAs interleaved 1:1 with the
                # pass-1 xT chunk loads on the SAME queue so the critical xT
                # stream is never starved by the resident loads).
                xres_tiles = [
                    itp.tile([P, CB, X2CH], BF16, tag=f"xr{i}", name=f"xr{i}")
                    for i in range(NX2)
                ]

                def load_xres(ch):
                    nc.sync.dma_start(
                        out=xres_tiles[ch],
                        in_=x_d[:, :, ch * X2CH : (ch + 1) * X2CH],
                    )

                # ---- pass 1: G = xT^T xT (PSUM-resident), xacc on DVE ---
                xacc = itp.tile([P, C], BF16, tag="xacc")
                tpsum_cm = tc.tile_pool(name="tps", bufs=2, space="PSUM")
                tpsum = tpsum_cm.__enter__()
                upsum_cm = tc.tile_pool(name="ups", bufs=1, space="PSUM")
                upsum = upsum_cm.__enter__()
                gpsum_cm = tc.tile_pool(name="gps", bufs=1, space="PSUM")
                gpsum = gpsum_cm.__enter__()

                with (
                    tc.tile_pool(name="xtp", bufs=6) as xtpool,
                ):
                    g_ps = gpsum.tile([P, CB, C], F32, tag="g")
                    xt_tiles = []
                    # all xT chunks first on the sync queue; the pass-2
                    # resident loads queue strictly behind them (FIFO), so
                    # they soak up bandwidth only once pass 1 is fed.
                    for ch in range(NXCH):
                        xt = xtpool.tile([P, SLABS, C], BF16, tag="xt")
                        xt_tiles.append(xt)
                        nc.sync.dma_start(
                            out=xt,
                            in_=xT_d[:, ch * SLABS : (ch + 1) * SLABS, :],
                        )
                    for ch in range(NX2):
                        load_xres(ch)
                    for ch in range(NXCH):
                        xt = xt_tiles[ch]
                        for s in range(SLABS):
                            first = ch == 0 and s == 0
                            last = ch == NXCH - 1 and s == SLABS - 1
                            # G symmetric: blocks (ci, cj>=ci) only
                            for ci in range(CB):
                                nc.tensor.matmul(
                                    g_ps[:, ci, ci * P :],
                                    xt[:, s, ci * P : (ci + 1) * P],
                                    xt[:, s, ci * P :],
                                    start=first,
                                    stop=last,
                                )
                            with nc.allow_low_precision(
                                reason="bf16 xacc: 0.4%% per add, feeds rank-1 "
                                "bias terms only (~0.1%% of S)"
                            ):
                                if first:
                                    nc.vector.tensor_copy(xacc, xt[:, 0, :])
                                else:
                                    nc.vector.tensor_add(xacc, xacc, xt[:, s, :])

                # Gp (bf16): 4 upper-block copies in A's consumption order
                # (row 0 first); lower blocks filled by PE transposes that
                # interleave with A's kd-steps below.
                Gp = itp.tile([P, CB, C], BF16, tag="gp")
                for ci in range(CB):
                    if ci % 2 == 0:
                        nc.scalar.copy(Gp[:, ci, ci * P :], g_ps[:, ci, ci * P :])
                    else:
                        with nc.allow_low_precision(reason="G to bf16"):
                            nc.vector.tensor_copy(
                                Gp[:, ci, ci * P :], g_ps[:, ci, ci * P :]
                            )
                gpsum_cm.__exit__(None, None, None)

                # xs_col[c] = sum_n x[c, n]: cross-partition reduce of
                # xacc via 4 tiny matmuls against a ones column.
                xs_t = upsum.tile([P, 2, C], F32, tag="u")
                for kc in range(CB):
                    nc.tensor.matmul(
                        xs_t[:, 0, kc : kc + 1],
                        xacc[:, kc * P : (kc + 1) * P],
                        ones_col,
                        start=True,
                        stop=True,
                    )
                xs_colb = itp.tile([P, CB], BF16, tag="xs_colb")
                with nc.allow_low_precision(reason="xs to bf16 operand"):
                    nc.vector.tensor_copy(xs_colb, xs_t[:, 0, 0:CB])

                # ---- chain: A, S, softmax||P1, zT, r --------------------
                E_sb = itp.tile([P, CB, C], BF16, tag="e")
                ssum = itp.tile([P, CB], F32, tag="ssum")
                rs = itp.tile([P, CB], F32, tag="rs")
                r_col = itp.tile([P, CB], F32, tag="r_col")
                zT = itp.tile([P, CB, C], BF16, tag="zT")

                if True:
                    # A = G Wk^T in fresh PSUM banks (no WAW against the G
                    # slot). kd-outer: row kd's lower blocks are produced by
                    # transposes interleaved just ahead of their use.
                    apool_cm = tc.tile_pool(name="aps", bufs=2, space="PSUM")
                    apool = apool_cm.__enter__()
                    aA_ps = apool.tile([P, 2, C], F32, tag="a")
                    aB_ps = apool.tile([P, 2, C], F32, tag="a")
                    Ap = itp.tile([P, CB, C], BF16, tag="ap")

                    def a_step(kd):
                        for ci in range(CB):
                            nc.tensor.matmul(
                                (aA_ps if ci < 2 else aB_ps)[:, ci % 2, :],
                                Gp[:, kd, ci * P : (ci + 1) * P],
                                wkT_sb[:, kd, :],
                                start=(kd == 0),
                                stop=(kd == CB - 1),
                            )

                    def lower_fill(b, a):
                        pst = tpsum.tile([P, P], BF16, tag="tp")
                        nc.tensor.transpose(
                            pst, Gp[:, b, a * P : (a + 1) * P], identb
                        )
                        if (a + b) % 2 == 0:
                            nc.scalar.copy(Gp[:, a, b * P : (b + 1) * P], pst)
                        else:
                            nc.vector.tensor_copy(
                                Gp[:, a, b * P : (b + 1) * P], pst
                            )

                    a_step(0)
                    lower_fill(0, 1)
                    a_step(1)
                    lower_fill(0, 2)
                    lower_fill(1, 2)
                    a_step(2)
                    lower_fill(0, 3)
                    lower_fill(1, 3)
                    lower_fill(2, 3)
                    a_step(3)

                    # u = Wk xs + N bk ; u2 = s*Wq xs: tiny matmuls that
                    # fill the PE while the Ap copies drain. Results land in
                    # the [2, C] stacks so both rank-1 S terms fuse into ONE
                    # rank-2 matmul per row-block:
                    #   S_bias[ci] = qs_stack[:, ci]^T @ ub_stack
                    #             = u2[ci] (x) bk  +  s*bq[ci] (x) u
                    u_row = itp.tile([1, C], BF16, tag="u_row")
                    u_ps = upsum.tile([P, 2, C], F32, tag="u")
                    for kc in range(CB):
                        nc.tensor.matmul(
                            u_ps[0:1, 0, :],
                            xs_colb[:, kc : kc + 1],
                            wkT_sb[:, kc, :],
                            start=(kc == 0),
                            stop=(kc == CB - 1),
                        )
                    for kc in range(CB):
                        nc.tensor.matmul(
                            u_ps[0:1, 1, :],
                            xs_colb[:, kc : kc + 1],
                            wqT_sb[:, kc, :],
                            start=(kc == 0),
                            stop=(kc == CB - 1),
                        )
                    # 4-way copies in S's consumption order (kc) so the
                    # first S matmul starts after the first part lands.
                    nc.scalar.copy(Ap[:, 0, :], aA_ps[:, 0, :])
                    with nc.allow_low_precision(reason="A to bf16"):
                        nc.vector.tensor_copy(Ap[:, 1, :], aA_ps[:, 1, :])
                    nc.scalar.copy(Ap[:, 2, :], aB_ps[:, 0, :])
                    with nc.allow_low_precision(reason="A to bf16"):
                        nc.vector.tensor_copy(Ap[:, 3, :], aB_ps[:, 1, :])
                    with nc.allow_low_precision(reason="bias rows to bf16"):
                        nc.vector.tensor_add(u_row, u_ps[0:1, 0, :], bkN_row)
                    nc.scalar.copy(qs_stack[0:1, :], u_ps[0:1, 1, :])
                    # engines can't address partition 1 alone; a tiny
                    # SBUF->SBUF DMA on the idle gpsimd queue moves u there
                    nc.gpsimd.dma_start(out=ub_stack[1:2, :], in_=u_row[0:1, :])
                    apool_cm.__exit__(None, None, None)
                    upsum_cm.__exit__(None, None, None)
                    tpsum_cm.__exit__(None, None, None)

                    # S = (s Wq) A + u2 bk^T + (s bq) u^T. One 1-bank PSUM
                    # tile per row-block so block ci+1's matmuls never wait
                    # on block ci's exp read; per-block exp -> P1 partials.
                    wpTs = itp.tile([P, CB, C], BF16, tag="wpts")
                    av_col = itp.tile([P, CB], F32, tag="av_col")
                    scr = itp.tile([P, C], F32, tag="scr")
                    p1_sb = itp.tile([P, CB, C], BF16, tag="p1")
                    spool_cm = tc.tile_pool(name="sps", bufs=4, space="PSUM")
                    spool = spool_cm.__enter__()
                    p1pool_cm = tc.tile_pool(name="p1ps", bufs=2, space="PSUM")
                    p1pool = p1pool_cm.__enter__()
                    p1a_ps = p1pool.tile([P, 2, C], F32, tag="p1")
                    p1b_ps = p1pool.tile([P, 2, C], F32, tag="p1")

                    def s_group(ci):
                        s_ps = spool.tile([P, C], F32, tag="s")
                        for kc in range(CB):
                            nc.tensor.matmul(
                                s_ps,
                                wqT_sb[:, kc, ci * P : (ci + 1) * P],
                                Ap[:, kc, :],
                                start=(kc == 0),
                                stop=False,
                            )
                        nc.tensor.matmul(
                            s_ps,
                            qs_stack[0:2, ci * P : (ci + 1) * P],
                            ub_stack,
                            start=False,
                            stop=True,
                        )
                        nc.scalar.activation(
                            E_sb[:, ci, :],
                            s_ps,
                            AF.Exp,
                            bias=0.0,
                            scale=1.0,
                            accum_out=ssum[:, ci : ci + 1],
                        )
                        # per-block: recip, scaled Wp^T row-block, av dot
                        nc.vector.reciprocal(
                            rs[:, ci : ci + 1], ssum[:, ci : ci + 1]
                        )
                        with nc.allow_low_precision(reason="bf16 scaled weights"):
                            nc.vector.tensor_scalar_mul(
                                wpTs[:, ci, :], wpT_sb[:, ci, :], rs[:, ci : ci + 1]
                            )
                        nc.vector.tensor_mul(scr, E_sb[:, ci, :], bv_bcast)
                        nc.vector.reduce_sum(av_col[:, ci : ci + 1], scr, axis=AX)

                    def p1_group(ci, half):
                        ps = p1a_ps if half == 0 else p1b_ps
                        for bd in (0, 1) if half == 0 else (2, 3):
                            nc.tensor.matmul(
                                ps[:, bd % 2, :],
                                E_sb[:, ci, bd * P : (bd + 1) * P],
                                wpTs[:, ci, :],
                                start=(ci == 0),
                                stop=(ci == CB - 1),
                            )

                    # S groups run back-to-back; each P1 group trails a full
                    # S group behind its exp, so the in-order PE queue never
                    # stalls on the softmax.
                    s_group(0)
                    s_group(1)
                    p1_group(0, 0)
                    p1_group(0, 1)
                    s_group(2)
                    p1_group(1, 0)
                    p1_group(1, 1)
                    s_group(3)
                    p1_group(2, 0)
                    p1_group(2, 1)
                    p1_group(3, 0)
                    p1_group(3, 1)
                    # copies in zT's consumption order (kd)
                    nc.scalar.copy(p1_sb[:, 0, :], p1a_ps[:, 0, :])
                    with nc.allow_low_precision(reason="P1 to bf16"):
                        nc.vector.tensor_copy(p1_sb[:, 2, :], p1b_ps[:, 0, :])
                    nc.scalar.copy(p1_sb[:, 1, :], p1a_ps[:, 1, :])
                    with nc.allow_low_precision(reason="P1 to bf16"):
                        nc.vector.tensor_copy(p1_sb[:, 3, :], p1b_ps[:, 1, :])

                    # r = Wp (rs*av) + bp (tiny matmuls fill the P1b copy)
                    avnb = itp.tile([P, CB], BF16, tag="avnb")
                    with nc.allow_low_precision(reason="bf16 rank-1 operand"):
                        nc.vector.tensor_mul(avnb, av_col, rs)
                    rp_ps = p1pool.tile([P, 2, C], F32, tag="p1")
                    for ob in range(CB):
                        for kc in range(CB):
                            nc.tensor.matmul(
                                rp_ps[:, 0, ob : ob + 1],
                                wpT_sb[:, kc, ob * P : (ob + 1) * P],
                                avnb[:, kc : kc + 1],
                                start=(kc == 0),
                                stop=(kc == CB - 1),
                            )
                    nc.vector.tensor_add(r_col, rp_ps[:, 0, 0:CB], bp_col)
                    p1pool_cm.__exit__(None, None, None)

                    # zT = Wv^T P1 (1-bank tiles from the S ring)
                    zt_tiles = []
                    for bj in range(CB):
                        zt_ps = spool.tile([P, C], F32, tag="s")
                        zt_tiles.append(zt_ps)
                        for kd in range(CB):
                            nc.tensor.matmul(
                                zt_ps,
                                wv_sb[:, kd, bj * P : (bj + 1) * P],
                                p1_sb[:, kd, :],
                                start=(kd == 0),
                                stop=(kd == CB - 1),
                            )
                        if bj % 2 == 0:
                            nc.scalar.copy(zT[:, bj, :], zt_ps)
                        else:
                            with nc.allow_low_precision(reason="zT to bf16"):
                                nc.vector.tensor_copy(zT[:, bj, :], zt_ps)

                    spool_cm.__exit__(None, None, None)

                # ---- pass 2: y = Z x + r --------------------------------
                with (
                    tc.tile_pool(name="ysb", bufs=3) as ysbpool,
                    tc.tile_pool(name="yps", bufs=2, space="PSUM") as ypsum,
                ):
                    for ch in range(NX2):
                        x_t = xres_tiles[ch]
                        for nb in range(X2CH // C):
                            y_sb = ysbpool.tile([P, CB, C], BF16, tag="ysb")
                            for half in range(2):
                                y_ps = ypsum.tile([P, 2, C], F32, tag="y")
                                for oh in range(2):
                                    ob = 2 * half + oh
                                    for kc in range(CB):
                                        nc.tensor.matmul(
                                            y_ps[:, oh, :],
                                            zT[:, kc, ob * P : (ob + 1) * P],
                                            x_t[:, kc, nb * C : (nb + 1) * C],
                                            start=(kc == 0),
                                            stop=(kc == CB - 1),
                                        )
                                for oh in range(2):
                                    ob = 2 * half + oh
                                    if ob % 2 == 0:
                                        nc.scalar.add(
                                            y_sb[:, ob, :],
                                            y_ps[:, oh, :],
                                            add=r_col[:, ob : ob + 1],
                                        )
                                    else:
                                        nc.vector.tensor_scalar_add(
                                            y_sb[:, ob, :],
                                            y_ps[:, oh, :],
                                            r_col[:, ob : ob + 1],
                                        )
                            n0 = ch * X2CH + nb * C
                            nc.sync.dma_start(
                                out=y_d[:, n0 : n0 + C].rearrange(
                                    "(kc p) n -> p kc n", p=P
                                ),
                                in_=y_sb,
                            )

                itp_cm.__exit__(None, None, None)

            for _it in range(repeat):
                if _it:
                    tc.strict_bb_all_engine_barrier()
                body(_it == 0)

    nc.compile()
    return nc


_NC = None


def _get_nc():
    global _NC
    if _NC is None:
        _NC = _build_nc()
    return _NC


def _make_in_maps(x, w_qkv, b_qkv, w_proj, b_proj):
    bf = ml_dtypes.bfloat16
    x = np.asarray(x, dtype=np.float32).reshape(B, C, N)
    w_qkv = np.asarray(w_qkv, dtype=np.float32)
    b_qkv = np.asarray(b_qkv, dtype=np.float32)
    w_proj = np.asarray(w_proj, dtype=np.float32)
    b_proj = np.asarray(b_proj, dtype=np.float32)
    Wq, Wk, Wv = w_qkv[:C], w_qkv[C : 2 * C], w_qkv[2 * C :]
    bq, bk, bv = b_qkv[:C], b_qkv[C : 2 * C], b_qkv[2 * C :]

    shared = {
        "wqT": np.ascontiguousarray((SCALE * Wq).T).astype(bf),
        "wkT": np.ascontiguousarray(Wk.T).astype(bf),
        "wpT": np.ascontiguousarray(w_proj.T).astype(bf),
        "wv": np.ascontiguousarray(Wv).astype(bf),
        "bqs": (SCALE * bq).reshape(1, C).astype(bf),
        "bk": bk.reshape(1, C).astype(bf),
        "bkN": (float(N) * bk).reshape(1, C).astype(np.float32),
        "bv": bv.reshape(1, C).astype(bf),
        "bp": b_proj.reshape(1, C).astype(np.float32),
    }
    maps = []
    for i in range(B):
        xi = x[i]
        # x2[p, kc, n] = x[kc*128+p, n] ; xT2[p, t, c] = x[c, t*128+p]
        x2 = np.ascontiguousarray(
            xi.reshape(CB, P, N).transpose(1, 0, 2)
        ).astype(bf)
        xT2 = np.ascontiguousarray(
            xi.T.reshape(N // P, P, C).transpose(1, 0, 2)
        ).astype(bf)
        maps.append({"x2": x2, "xT2": xT2, **shared})
    return maps


def run_sharded(x, w_qkv, b_qkv, w_proj, b_proj, trace=False, **kwargs):
    nc = _get_nc()
    in_maps = _make_in_maps(x, w_qkv, b_qkv, w_proj, b_proj)
    res = run_bass_kernel_spmd(nc, in_maps, core_ids=list(range(B)), trace=trace, **kwargs)
    y = np.stack(
        [res.results[i]["y"].astype(np.float32) for i in range(B)]
    ).reshape(B, C, H, W)
    return y, res


def _clear_devices():
    """Run a trivial kernel to flush any wedged device state left by a
    previously-crashed NEFF (NRT_EXEC_UNIT_UNRECOVERABLE is sometimes sticky
    for exactly one subsequent launch)."""
    nc = bacc.Bacc("TRN2", target_bir_lowering=False, debug=False, num_devices=B)
    xi = nc.declare_dram_parameter("xi", [P, P], F32, isOutput=False)
    yo = nc.declare_dram_parameter("yo", [P, P], F32, isOutput=True)
    with tile.TileContext(nc) as tc:
        with tc.tile_pool(name="p", bufs=1) as pool:
            t = pool.tile([P, P], F32)
            nc.sync.dma_start(out=t, in_=xi[:, :])
            nc.sync.dma_start(out=yo[:, :], in_=t)
    nc.compile()
    z = np.zeros((P, P), np.float32)
    run_bass_kernel_spmd(nc, [{"xi": z} for _ in range(B)], core_ids=list(range(B)))


def _clear_devices_subprocess():
    # A wedged device sometimes only recovers for a FRESH PJRT client;
    # run the clearing kernel in a subprocess.
    import subprocess
    import sys

    subprocess.run(
        [sys.executable, "-c", "import kernel; kernel._clear_devices()"],
        timeout=600,
        cwd=_os.path.dirname(_os.path.abspath(__file__)) or ".",
    )


def kernel(x, w_qkv, b_qkv, w_proj, b_proj):
    import time as _time

    last = None
    for attempt in range(4):
        if attempt:
            _time.sleep(3.0 * attempt)
            try:
                if attempt >= 2:
                    _clear_devices_subprocess()
                else:
                    _clear_devices()
            except Exception:
                _time.sleep(5.0)
        try:
            y, _ = run_sharded(x, w_qkv, b_qkv, w_proj, b_proj, trace=False)
            return y
        except Exception as e:  # wedged device from a prior crashed NEFF
            last = e
    raise last


# revision 50
# speedup vs baseline: 1.1253x; 1.1253x over previous
"""Channel-attention block (qkv 1x1 conv -> C x C channel attention -> proj)
for Trainium2, data-parallel over batch across 8 NeuronCores.

Math (per batch element, x: [C, N] with C=512, N=9216):
  q = Wq x + bq 1^T,  k = Wk x + bk 1^T,  v = Wv x + bv 1^T
  S = (q k^T) / sqrt(C);  attn = softmax_rows(S);  y = Wp (attn v) + bp 1^T

Gram-matrix factorisation (~4.5 GFLOP/core instead of 29):
  G  = x x^T, xs = x 1
  S  = s*Wq G Wk^T + (s*Wq xs) bk^T + (s*bq)(Wk xs + N bk)^T
  E  = exp(S),  rs = 1/(E 1)
  Z  = Wp diag(rs) E Wv,  r = Wp (diag(rs) E bv) + bp
  y  = Z x + r 1^T

Design (the kernel is PE-stream-bound; everything else hides under it):
  * all matmuls in bf16 (fp32 PSUM): fixes the fp32r N<256 4x penalty,
    halves LDWEIGHTS (FWL), halves DMA; rel-err ~8e-3 vs the 2e-2 gate.
  * host uploads BOTH layouts of x, pre-tiled for 2-8KB-contiguous DMA
    lines: xT2 [128, N/128, C] feeds pass-1's G = x x^T directly (zero
    on-device transposes), x2 [128, CB, N] is SBUF-resident for pass 2.
  * one input DMA queue, ordered by criticality: 9 xT chunks first, the
    9 resident-x chunks strictly behind them (FIFO gating).
  * xs = x@1 via DVE-accumulated xT slabs + 4 tiny PE matmuls against a
    ones column (the only cross-partition reduce available).
  * chain is scheduled for the in-order PE queue + HAM clock-gate: G's
    lower-triangle transposes interleave with A's kd-steps, the two
    rank-1 S bias terms fuse into one rank-2 matmul per block, each P1
    group trails one S group behind its exp, r-matmuls cover the P1
    copies, and every GEMM handoff uses split PSUM pools (no WAW on a
    shared slot) with PSUM->SBUF copies split in consumption order.
  * y is stored bf16 and upcast on host.
Measured: ~110 us/iteration vs 202 us for the fp32r baseline (same
paired-slope harness), ~95% PE-pipe occupancy.
"""

import os as _os

import numpy as np
import ml_dtypes

import concourse.bass as bass
import concourse.bacc as bacc
import concourse.mybir as mybir
import concourse.tile as tile
from concourse.bass_utils import run_bass_kernel_spmd
from concourse.masks import make_identity

F32 = mybir.dt.float32
BF16 = mybir.dt.bfloat16
AX = mybir.AxisListType.X
AF = mybir.ActivationFunctionType

B = 8
C = 512
H = W = 96
N = H * W          # 9216
P = 128
CB = C // P        # 4 channel blocks
SCALE = 1.0 / float(np.sqrt(C))

XCH = 1024         # xT rows per pass-1 chunk
NXCH = N // XCH    # 9
SLABS = XCH // P   # 8
X2CH = 1024        # x cols per resident chunk
NX2 = N // X2CH    # 9


def _build_nc(repeat=1):
    nc = bacc.Bacc("TRN2", target_bir_lowering=False, debug=False, num_devices=B)

    # host-pretiled layouts: xT2[p, t, c] = x[c, t*128+p] (8KB contiguous
    # per partition per chunk), x2[p, kc, n] = x[kc*128+p, n] (2KB lines).
    xT_d = nc.declare_dram_parameter("xT2", [P, N // P, C], BF16, isOutput=False)
    x_d = nc.declare_dram_parameter("x2", [P, CB, N], BF16, isOutput=False)
    wqT_d = nc.declare_dram_parameter("wqT", [C, C], BF16, isOutput=False)
    wkT_d = nc.declare_dram_parameter("wkT", [C, C], BF16, isOutput=False)
    wpT_d = nc.declare_dram_parameter("wpT", [C, C], BF16, isOutput=False)
    wv_d = nc.declare_dram_parameter("wv", [C, C], BF16, isOutput=False)
    bqs_d = nc.declare_dram_parameter("bqs", [1, C], BF16, isOutput=False)
    bk_d = nc.declare_dram_parameter("bk", [1, C], BF16, isOutput=False)
    bkN_d = nc.declare_dram_parameter("bkN", [1, C], F32, isOutput=False)
    bv_d = nc.declare_dram_parameter("bv", [1, C], BF16, isOutput=False)
    bp_d = nc.declare_dram_parameter("bp", [1, C], F32, isOutput=False)
    y_d = nc.declare_dram_parameter("y", [C, N], BF16, isOutput=True)

    with tile.TileContext(nc) as tc:
        with (
            tc.tile_pool(name="consts", bufs=1) as consts,
        ):
            # ---- constants (loaded once, persist across repeats) --------
            # all on the gpsimd queue so the sync queue starts streaming
            # xT chunk 0 immediately at iteration start.
            wqT_sb = consts.tile([P, CB, C], BF16)
            nc.gpsimd.dma_start(
                out=wqT_sb, in_=wqT_d.rearrange("(kc p) i -> p kc i", p=P)
            )
            wkT_sb = consts.tile([P, CB, C], BF16)
            nc.gpsimd.dma_start(
                out=wkT_sb, in_=wkT_d.rearrange("(kc p) i -> p kc i", p=P)
            )
            wpT_sb = consts.tile([P, CB, C], BF16)
            nc.gpsimd.dma_start(
                out=wpT_sb, in_=wpT_d.rearrange("(kc p) i -> p kc i", p=P)
            )
            wv_sb = consts.tile([P, CB, C], BF16)
            nc.gpsimd.dma_start(
                out=wv_sb, in_=wv_d.rearrange("(kd p) a -> p kd a", p=P)
            )
            # rank-2 bias stacks: qs = (u2, s*bq), ub = (bk, u); constant
            # rows land once via DMA (engines can't write partition 1),
            # per-iteration rows are written below.
            qs_stack = consts.tile([2, C], BF16)
            nc.gpsimd.dma_start(out=qs_stack[1:2, :], in_=bqs_d[0:1, :])
            ub_stack = consts.tile([2, C], BF16)
            nc.gpsimd.dma_start(out=ub_stack[0:1, :], in_=bk_d[0:1, :])
            bkN_row = consts.tile([1, C], F32)
            nc.gpsimd.dma_start(out=bkN_row, in_=bkN_d[0:1, :])
            bv_bcast = consts.tile([P, C], BF16)
            _src = bv_d[0:1, :]
            nc.gpsimd.dma_start(
                out=bv_bcast,
                in_=bass.AP(tensor=_src.tensor, offset=_src.offset,
                            ap=[[0, P]] + _src.ap[1:]),
            )
            bp_col = consts.tile([P, CB], F32)
            nc.gpsimd.dma_start(
                out=bp_col, in_=bp_d[0:1, :].rearrange("o (kc p) -> (o p) kc", p=P)
            )
            ident32 = consts.tile([P, P], F32)
            make_identity(nc, ident32)
            identb = consts.tile([P, P], BF16)
            nc.vector.tensor_copy(identb, ident32)
            ones_col = consts.tile([P, 1], BF16)
            nc.vector.memset(ones_col, 1.0)

            def body(first_iter):
                itp_cm = tc.tile_pool(name="iter", bufs=1)
                itp = itp_cm.__enter__()

                # resident x tiles for pass 2 (DMAs interleaved 1:1 with the
                # pass-1 xT chunk loads on the SAME queue so the critical xT
                # stream is never starved by the resident loads).
                xres_tiles = [
                    itp.tile([P, CB, X2CH], BF16, tag=f"xr{i}", name=f"xr{i}")
                    for i in range(NX2)
                ]

                def load_xres(ch):
                    nc.sync.dma_start(
                        out=xres_tiles[ch],
                        in_=x_d[:, :, ch * X2CH : (ch + 1) * X2CH],
                    )

                # ---- pass 1: G = xT^T xT (PSUM-resident), xacc on DVE ---
                xacc = itp.tile([P, C], BF16, tag="xacc")
                tpsum_cm = tc.tile_pool(name="tps", bufs=2, space="PSUM")
                tpsum = tpsum_cm.__enter__()
                upsum_cm = tc.tile_pool(name="ups", bufs=1, space="PSUM")
                upsum = upsum_cm.__enter__()
                gpsum_cm = tc.tile_pool(name="gps", bufs=1, space="PSUM")
                gpsum = gpsum_cm.__enter__()

                with (
                    tc.tile_pool(name="xtp", bufs=6) as xtpool,
                ):
                    g_ps = gpsum.tile([P, CB, C], F32, tag="g")
                    xt_tiles = []
                    # all xT chunks first on the sync queue; the pass-2
                    # resident loads queue strictly behind them (FIFO), so
                    # they soak up bandwidth only once pass 1 is fed.
                    for ch in range(NXCH):
                        xt = xtpool.tile([P, SLABS, C], BF16, tag="xt")
                        xt_tiles.append(xt)
                        nc.sync.dma_start(
                            out=xt,
                            in_=xT_d[:, ch * SLABS : (ch + 1) * SLABS, :],
                        )
                    for ch in range(NX2):
                        load_xres(ch)
                    for ch in range(NXCH):
                        xt = xt_tiles[ch]
                        for s in range(SLABS):
                            first = ch == 0 and s == 0
                            last = ch == NXCH - 1 and s == SLABS - 1
                            # G symmetric: blocks (ci, cj>=ci) only
                            for ci in range(CB):
                                nc.tensor.matmul(
                                    g_ps[:, ci, ci * P :],
                                    xt[:, s, ci * P : (ci + 1) * P],
                                    xt[:, s, ci * P :],
                                    start=first,
                                    stop=last,
                                )
                            with nc.allow_low_precision(
                                reason="bf16 xacc: 0.4%% per add, feeds rank-1 "
                                "bias terms only (~0.1%% of S)"
                            ):
                                if first:
                                    nc.vector.tensor_copy(xacc, xt[:, 0, :])
                                else:
                                    nc.vector.tensor_add(xacc, xacc, xt[:, s, :])

                # Gp (bf16): 4 upper-block copies in A's consumption order
                # (row 0 first); lower blocks filled by PE transposes that
                # interleave with A's kd-steps below.
                Gp = itp.tile([P, CB, C], BF16, tag="gp")
                for ci in range(CB):
                    if ci % 2 == 0:
                        nc.scalar.copy(Gp[:, ci, ci * P :], g_ps[:, ci, ci * P :])
                    else:
                        with nc.allow_low_precision(reason="G to bf16"):
                            nc.vector.tensor_copy(
                                Gp[:, ci, ci * P :], g_ps[:, ci, ci * P :]
                            )
                gpsum_cm.__exit__(None, None, None)

                # xs_col[c] = sum_n x[c, n]: cross-partition reduce of
                # xacc via 4 tiny matmuls against a ones column.
                xs_t = upsum.tile([P, 2, C], F32, tag="u")
                for kc in range(CB):
                    nc.tensor.matmul(
                        xs_t[:, 0, kc : kc + 1],
                        xacc[:, kc * P : (kc + 1) * P],
                        ones_col,
                        start=True,
                        stop=True,
                    )
                xs_colb = itp.tile([P, CB], BF16, tag="xs_colb")
                with nc.allow_low_precision(reason="xs to bf16 operand"):
                    nc.vector.tensor_copy(xs_colb, xs_t[:, 0, 0:CB])

                # ---- chain: A, S, softmax||P1, zT, r --------------------
                E_sb = itp.tile([P, CB, C], BF16, tag="e")
                ssum = itp.tile([P, CB], F32, tag="ssum")
                rs = itp.tile([P, CB], F32, tag="rs")
                r_col = itp.tile([P, CB], F32, tag="r_col")
                zT = itp.tile([P, CB, C], BF16, tag="zT")

                if True:
                    # A = G Wk^T in fresh PSUM banks (no WAW against the G
                    # slot). kd-outer: row kd's lower blocks are produced by
                    # transposes interleaved just ahead of their use.
                    apool_cm = tc.tile_pool(name="aps", bufs=2, space="PSUM")
                    apool = apool_cm.__enter__()
                    aA_ps = apool.tile([P, 2, C], F32, tag="a")
                    aB_ps = apool.tile([P, 2, C], F32, tag="a")
                    Ap = itp.tile([P, CB, C], BF16, tag="ap")

                    def a_step(kd):
                        for ci in range(CB):
                            nc.tensor.matmul(
                                (aA_ps if ci < 2 else aB_ps)[:, ci % 2, :],
                                Gp[:, kd, ci * P : (ci + 1) * P],
                                wkT_sb[:, kd, :],
                                start=(kd == 0),
                                stop=(kd == CB - 1),
                            )

                    def lower_fill(b, a):
                        pst = tpsum.tile([P, P], BF16, tag="tp")
                        nc.tensor.transpose(
                            pst, Gp[:, b, a * P : (a + 1) * P], identb
                        )
                        if (a + b) % 2 == 0:
                            nc.scalar.copy(Gp[:, a, b * P : (b + 1) * P], pst)
                        else:
                            nc.vector.tensor_copy(
                                Gp[:, a, b * P : (b + 1) * P], pst
                            )

                    a_step(0)
                    lower_fill(0, 1)
                    a_step(1)
                    lower_fill(0, 2)
                    lower_fill(1, 2)
                    a_step(2)
                    lower_fill(0, 3)
                    lower_fill(1, 3)
                    lower_fill(2, 3)
                    a_step(3)

                    # u = Wk xs + N bk ; u2 = s*Wq xs: tiny matmuls that
                    # fill the PE while the Ap copies drain. Results land in
                    # the [2, C] stacks so both rank-1 S terms fuse into ONE
                    # rank-2 matmul per row-block:
                    #   S_bias[ci] = qs_stack[:, ci]^T @ ub_stack
                    #             = u2[ci] (x) bk  +  s*bq[ci] (x) u
                    u_row = itp.tile([1, C], BF16, tag="u_row")
                    u_ps = upsum.tile([P, 2, C], F32, tag="u")
                    for kc in range(CB):
                        nc.tensor.matmul(
                            u_ps[0:1, 0, :],
                            xs_colb[:, kc : kc + 1],
                            wkT_sb[:, kc, :],
                            start=(kc == 0),
                            stop=(kc == CB - 1),
                        )
                    for kc in range(CB):
                        nc.tensor.matmul(
                            u_ps[0:1, 1, :],
                            xs_colb[:, kc : kc + 1],
                            wqT_sb[:, kc, :],
                            start=(kc == 0),
                            stop=(kc == CB - 1),
                        )
                    # 4-way copies in S's consumption order (kc) so the
                    # first S matmul starts after the first part lands.
                    nc.scalar.copy(Ap[:, 0, :], aA_ps[:, 0, :])
                    with nc.allow_low_precision(reason="A to bf16"):
                        nc.vector.tensor_copy(Ap[:, 1, :], aA_ps[:, 1, :])
                    nc.scalar.copy(Ap[:, 2, :], aB_ps[:, 0, :])
                    with nc.allow_low_precision(reason="A to bf16"):
                        nc.vector.tensor_copy(Ap[:, 3, :], aB_ps[:, 1, :])
                    with nc.allow_low_precision(reason="bias rows to bf16"):
                        nc.vector.tensor_add(u_row, u_ps[0:1, 0, :], bkN_row)
                    nc.scalar.copy(qs_stack[0:1, :], u_ps[0:1, 1, :])
                    # engines can't address partition 1 alone; a tiny
                    # SBUF->SBUF DMA on the idle gpsimd queue moves u there
                    nc.gpsimd.dma_start(out=ub_stack[1:2, :], in_=u_row[0:1, :])
                    apool_cm.__exit__(None, None, None)
                    upsum_cm.__exit__(None, None, None)
                    tpsum_cm.__exit__(None, None, None)

                    # S = (s Wq) A + u2 bk^T + (s bq) u^T. One 1-bank PSUM
                    # tile per row-block so block ci+1's matmuls never wait
                    # on block ci's exp read; per-block exp -> P1 partials.
                    wpTs = itp.tile([P, CB, C], BF16, tag="wpts")
                    av_col = itp.tile([P, CB], F32, tag="av_col")
                    scr = itp.tile([P, C], F32, tag="scr")
                    p1_sb = itp.tile([P, CB, C], BF16, tag="p1")
                    spool_cm = tc.tile_pool(name="sps", bufs=4, space="PSUM")
                    spool = spool_cm.__enter__()
                    p1pool_cm = tc.tile_pool(name="p1ps", bufs=2, space="PSUM")
                    p1pool = p1pool_cm.__enter__()
                    p1a_ps = p1pool.tile([P, 2, C], F32, tag="p1")
                    p1b_ps = p1pool.tile([P, 2, C], F32, tag="p1")

                    def s_group(ci):
                        s_ps = spool.tile([P, C], F32, tag="s")
                        for kc in range(CB):
                            nc.tensor.matmul(
                                s_ps,
                                wqT_sb[:, kc, ci * P : (ci + 1) * P],
                                Ap[:, kc, :],
                                start=(kc == 0),
                                stop=False,
                            )
                        nc.tensor.matmul(
                            s_ps,
                            qs_stack[0:2, ci * P : (ci + 1) * P],
                            ub_stack,
                            start=False,
                            stop=True,
                        )
                        nc.scalar.activation(
                            E_sb[:, ci, :],
                            s_ps,
                            AF.Exp,
                            bias=0.0,
                            scale=1.0,
                            accum_out=ssum[:, ci : ci + 1],
                        )
                        # per-block: recip, scaled Wp^T row-block, av dot
                        nc.vector.reciprocal(
                            rs[:, ci : ci + 1], ssum[:, ci : ci + 1]
                        )
                        with nc.allow_low_precision(reason="bf16 scaled weights"):
                            nc.vector.tensor_scalar_mul(
                                wpTs[:, ci, :], wpT_sb[:, ci, :], rs[:, ci : ci + 1]
                            )
                        nc.vector.tensor_mul(scr, E_sb[:, ci, :], bv_bcast)
                        nc.vector.reduce_sum(av_col[:, ci : ci + 1], scr, axis=AX)

                    def p1_group(ci, half):
                        ps = p1a_ps if half == 0 else p1b_ps
                        for bd in (0, 1) if half == 0 else (2, 3):
                            nc.tensor.matmul(
                                ps[:, bd % 2, :],
                                E_sb[:, ci, bd * P : (bd + 1) * P],
                                wpTs[:, ci, :],
                                start=(ci == 0),
                                stop=(ci == CB - 1),
                            )

                    # S groups run back-to-back; each P1 group trails a full
                    # S group behind its exp, so the in-order PE queue never
                    # stalls on the softmax.
                    s_group(0)
                    s_group(1)
                    p1_group(0, 0)
                    p1_group(0, 1)
                    s_group(2)
                    p1_group(1, 0)
                    p1_group(1, 1)
                    s_group(3)
                    p1_group(2, 0)
                    p1_group(2, 1)
                    p1_group(3, 0)
                    p1_group(3, 1)
                    # copies in zT's consumption order (kd)
                    nc.scalar.copy(p1_sb[:, 0, :], p1a_ps[:, 0, :])
                    with nc.allow_low_precision(reason="P1 to bf16"):
                        nc.vector.tensor_copy(p1_sb[:, 2, :], p1b_ps[:, 0, :])
                    nc.scalar.copy(p1_sb[:, 1, :], p1a_ps[:, 1, :])
                    with nc.allow_low_precision(reason="P1 to bf16"):
                        nc.vector.tensor_copy(p1_sb[:, 3, :], p1b_ps[:, 1, :])

                    # r = Wp (rs*av) + bp (tiny matmuls fill the P1b copy)
                    avnb = itp.tile([P, CB], BF16, tag="avnb")
                    with nc.allow_low_precision(reason="bf16 rank-1 operand"):
                        nc.vector.tensor_mul(avnb, av_col, rs)
                    rp_ps = p1pool.tile([P, 2, C], F32, tag="p1")
                    for ob in range(CB):
                        for kc in range(CB):
                            nc.tensor.matmul(
                                rp_ps[:, 0, ob : ob + 1],
                                wpT_sb[:, kc, ob * P : (ob + 1) * P],
                                avnb[:, kc : kc + 1],
                                start=(kc == 0),
                                stop=(kc == CB - 1),
                            )
                    nc.vector.tensor_add(r_col, rp_ps[:, 0, 0:CB], bp_col)
                    p1pool_cm.__exit__(None, None, None)

                    # zT = Wv^T P1 (1-bank tiles from the S ring)
                    zt_tiles = []
                    for bj in range(CB):
                        zt_ps = spool.tile([P, C], F32, tag="s")
                        zt_tiles.append(zt_ps)
                        for kd in range(CB):
                            nc.tensor.matmul(
                                zt_ps,
                                wv_sb[:, kd, bj * P : (bj + 1) * P],
                                p1_sb[:, kd, :],
                                start=(kd == 0),
                                stop=(kd == CB - 1),
                            )
                        if bj % 2 == 0:
                            nc.scalar.copy(zT[:, bj, :], zt_ps)
                        else:
                            with nc.allow_low_precision(reason="zT to bf16"):
                                nc.vector.tensor_copy(zT[:, bj, :], zt_ps)

                    spool_cm.__exit__(None, None, None)

                # ---- pass 2: y = Z x + r --------------------------------
                with (
                    tc.tile_pool(name="ysb", bufs=4) as ysbpool,
                    tc.tile_pool(name="yps", bufs=3, space="PSUM") as ypsum,
                ):
                    for ch in range(NX2):
                        x_t = xres_tiles[ch]
                        for nb in range(X2CH // C):
                            y_sb = ysbpool.tile([P, CB, C], BF16, tag="ysb")
                            for half in range(2):
                                y_ps = ypsum.tile([P, 2, C], F32, tag="y")
                                for oh in range(2):
                                    ob = 2 * half + oh
                                    for kc in range(CB):
                                        nc.tensor.matmul(
                                            y_ps[:, oh, :],
                                            zT[:, kc, ob * P : (ob + 1) * P],
                                            x_t[:, kc, nb * C : (nb + 1) * C],
                                            start=(kc == 0),
                                            stop=(kc == CB - 1),
                                        )
                                for oh in range(2):
                                    ob = 2 * half + oh
                                    if ob % 2 == 0:
                                        nc.scalar.add(
                                            y_sb[:, ob, :],
                                            y_ps[:, oh, :],
                                            add=r_col[:, ob : ob + 1],
                                        )
                                    else:
                                        nc.vector.tensor_scalar_add(
                                            y_sb[:, ob, :],
                                            y_ps[:, oh, :],
                                            r_col[:, ob : ob + 1],
                                        )
                            n0 = ch * X2CH + nb * C
                            nc.sync.dma_start(
                                out=y_d[:, n0 : n0 + C].rearrange(
                                    "(kc p) n -> p kc n", p=P
                                ),
                                in_=y_sb,
                            )

                itp_cm.__exit__(None, None, None)

            for _it in range(repeat):
                if _it:
                    tc.strict_bb_all_engine_barrier()
                body(_it == 0)

    nc.compile()
    return nc


_NC = None


def _get_nc():
    global _NC
    if _NC is None:
        _NC = _build_nc()
    return _NC


def _make_in_maps(x, w_qkv, b_qkv, w_proj, b_proj):
    bf = ml_dtypes.bfloat16
    x = np.asarray(x, dtype=np.float32).reshape(B, C, N)
    w_qkv = np.asarray(w_qkv, dtype=np.float32)
    b_qkv = np.asarray(b_qkv, dtype=np.float32)
    w_proj = np.asarray(w_proj, dtype=np.float32)
    b_proj = np.asarray(b_proj, dtype=np.float32)
    Wq, Wk, Wv = w_qkv[:C], w_qkv[C : 2 * C], w_qkv[2 * C :]
    bq, bk, bv = b_qkv[:C], b_qkv[C : 2 * C], b_qkv[2 * C :]

    shared = {
        "wqT": np.ascontiguousarray((SCALE * Wq).T).astype(bf),
        "wkT": np.ascontiguousarray(Wk.T).astype(bf),
        "wpT": np.ascontiguousarray(w_proj.T).astype(bf),
        "wv": np.ascontiguousarray(Wv).astype(bf),
        "bqs": (SCALE * bq).reshape(1, C).astype(bf),
        "bk": bk.reshape(1, C).astype(bf),
        "bkN": (float(N) * bk).reshape(1, C).astype(np.float32),
        "bv": bv.reshape(1, C).astype(bf),
        "bp": b_proj.reshape(1, C).astype(np.float32),
    }
    maps = []
    for i in range(B):
        xi = x[i]
        # x2[p, kc, n] = x[kc*128+p, n] ; xT2[p, t, c] = x[c, t*128+p]
        x2 = np.ascontiguousarray(
            xi.reshape(CB, P, N).transpose(1, 0, 2)
        ).astype(bf)
        xT2 = np.ascontiguousarray(
            xi.T.reshape(N // P, P, C).transpose(1, 0, 2)
        ).astype(bf)
        maps.append({"x2": x2, "xT2": xT2, **shared})
    return maps


def run_sharded(x, w_qkv, b_qkv, w_proj, b_proj, trace=False, **kwargs):
    nc = _get_nc()
    in_maps = _make_in_maps(x, w_qkv, b_qkv, w_proj, b_proj)
    res = run_bass_kernel_spmd(nc, in_maps, core_ids=list(range(B)), trace=trace, **kwargs)
    y = np.stack(
        [res.results[i]["y"].astype(np.float32) for i in range(B)]
    ).reshape(B, C, H, W)
    return y, res


def _clear_devices():
    """Run a trivial kernel to flush any wedged device state left by a
    previously-crashed NEFF (NRT_EXEC_UNIT_UNRECOVERABLE is sometimes sticky
    for exactly one subsequent launch)."""
    nc = bacc.Bacc("TRN2", target_bir_lowering=False, debug=False, num_devices=B)
    xi = nc.declare_dram_parameter("xi", [P, P], F32, isOutput=False)
    yo = nc.declare_dram_parameter("yo", [P, P], F32, isOutput=True)
    with tile.TileContext(nc) as tc:
        with tc.tile_pool(name="p", bufs=1) as pool:
            t = pool.tile([P, P], F32)
            nc.sync.dma_start(out=t, in_=xi[:, :])
            nc.sync.dma_start(out=yo[:, :], in_=t)
    nc.compile()
    z = np.zeros((P, P), np.float32)
    run_bass_kernel_spmd(nc, [{"xi": z} for _ in range(B)], core_ids=list(range(B)))


def _clear_devices_subprocess():
    # A wedged device sometimes only recovers for a FRESH PJRT client;
    # run the clearing kernel in a subprocess.
    import subprocess
    import sys

    subprocess.run(
        [sys.executable, "-c", "import kernel; kernel._clear_devices()"],
        timeout=600,
        cwd=_os.path.dirname(_os.path.abspath(__file__)) or ".",
    )


def kernel(x, w_qkv, b_qkv, w_proj, b_proj):
    import time as _time

    last = None
    for attempt in range(4):
        if attempt:
            _time.sleep(3.0 * attempt)
            try:
                if attempt >= 2:
                    _clear_devices_subprocess()
                else:
                    _clear_devices()
            except Exception:
                _time.sleep(5.0)
        try:
            y, _ = run_sharded(x, w_qkv, b_qkv, w_proj, b_proj, trace=False)
            return y
        except Exception as e:  # wedged device from a prior crashed NEFF
            last = e
    raise last
